# revision 14
# baseline (speedup 1.0000x reference)
"""Equivariant matmul kernel for Trainium2 (8 NeuronCores, Bass/Tile).

Problem (per edge e of E=800000):
    out[e,o,m] = (sum_i basis[e,o,i] * node_features[U[e],i,m]) * w[e,lo(o),m]
with D_IN=D_OUT=4, M=32, lo = [0,1,1,1].

Strategy (edge-parallel sharding, 100k edges/core, fp16 data / fp32 PSUM):
- Host prep per core shard (196 superblocks x 512 edges, batched 7 SBs
  per DMA round -> 28 batches):
    * x_arr[nb, 4b+i, 512*sb + 32g+m] = nf[U[e],i,m]  (gather, fp16)
    * w_arr[nb, 32j+m, 256*sb + 64c+2b+l] = edge_weights (compact, fp16)
    * bsrc[nb, 4b+i, 448b-relative run (o,g,sb)] = basis^T payload (fp16)
- Device per batch:
    * one plain DMA each for x / w / payload / out; the payload lands in
      a persistent pre-zeroed block-diagonal tensor bd[4b+i, 448b+112o+
      7g+sb] via a partition-crossing strided AP (512B+ contiguous runs)
    * per superblock 16 fp16 TensorE matmuls (K=128, M=32, N=128,
      4x col-tiled) read bd through a strided AP; PSUM accumulates fp32
    * 2 DVE tensor_muls apply the radial weights (stride-0 broadcast
      expands the l=1 weight over o in {1,2,3}) draining PSUM->SBUF fp16
- Host post: invert the layout permutation, cast fp32, concat shards.
"""

import contextlib
import ctypes
import sys
import types

import numpy as np

# ---------------------------------------------------------------- harness
# Workaround for walrus "Too many sync wait commands": this container's
# compiler accepts at most MAXW sem-waits per instruction; Tile emits more
# on the tail drain and occasionally mid-kernel. Split extras onto NOPs.
MAXW = 1


def _apply_tile_patch():
    import concourse.tile as tile_mod
    import concourse.mybir as mb
    from concourse.vector_clock import ScopedClock

    def _patched_drain_and_barrier(self, tick_clock, wait_clock):
        nc = self.nc
        drain_inst = nc.sync.drain()
        wait_clock.add_sem_waits(
            drain_inst.ins, ScopedClock({None: tick_clock.global_clock})
        )
        si = drain_inst.ins.sync_info
        if si is not None and len(si.on_wait) > 1:
            extra = list(si.on_wait[1:])
            si.on_wait = si.on_wait[:1]
            for w in extra:
                nop = nc.sync.nop(nofuse=True, hint="split_drain_wait")
                nop.ins.sync_info = mb.SyncInfo(on_wait=[w], on_update=[])
        nc.all_engine_barrier()
        assert self.sems is not None
        popped = nc._tile_sem_poison_stack.pop()
        assert popped is self._sem_poison
        nc.clear_and_free_semaphores(list(self.sems.allocated().values()))
        nc.all_engine_barrier()

    tile_mod.TileContext._drain_and_barrier = _patched_drain_and_barrier


_nop_counter = [0]


def _split_waits(nc, maxw=MAXW):
    import concourse.mybir as mb

    n_split = 0
    for fn in nc.m.functions:
        for blk in fn.blocks:
            insts = list(blk.instructions)
            out = []
            changed = False
            for inst in insts:
                si = getattr(inst, "sync_info", None)
                if si is not None and si.on_wait is not None and len(si.on_wait) > maxw:
                    extra = list(si.on_wait[:-maxw])
                    si.on_wait = list(si.on_wait[-maxw:])
                    for w in extra:
                        _nop_counter[0] += 1
                        nop = mb.InstNoOp(
                            name=f"waitsplit-{_nop_counter[0]}",
                            ins=[], outs=[], engine=inst.engine,
                        )
                        nop.sync_info = mb.SyncInfo(on_wait=[w], on_update=[])
                        out.append(nop)
                        n_split += 1
                    changed = True
                out.append(inst)
            if changed:
                blk.instructions = out
    return n_split


def _install_axon_ntff_hook():
    """Register the NTFF profile hook the agent image's antenv lacks, so
    run_bass_kernel_spmd(trace=True) can report HW exec time."""
    if "antenv.axon_hooks" in sys.modules:
        return
    so_path = "/opt/axon/libaxon_pjrt.so"
    holder = {}

    def _make_hook():
        try:
            lib = ctypes.CDLL(so_path)
        except OSError:
            return None
        if not hasattr(lib, "axon_start_nrt_profile"):
            return None
        lib.axon_start_nrt_profile.argtypes = [
            ctypes.POINTER(ctypes.c_int64), ctypes.c_size_t,
        ]
        lib.axon_start_nrt_profile.restype = ctypes.c_int64
        lib.axon_stop_nrt_profile.argtypes = [ctypes.c_char_p]
        lib.axon_stop_nrt_profile.restype = ctypes.c_int64

        @contextlib.contextmanager
        def _hook(output_dir, device_ids):
            import jax

            jax.devices()
            if device_ids:
                ids = (ctypes.c_int64 * len(device_ids))(*device_ids)
                rc = lib.axon_start_nrt_profile(ids, len(device_ids))
            else:
                rc = lib.axon_start_nrt_profile(None, 0)
            if rc != 0:
                raise RuntimeError(f"axon_start_nrt_profile rc={rc}")
            try:
                yield
            finally:
                n = lib.axon_stop_nrt_profile(str(output_dir).encode())
                if n < 0:
                    raise RuntimeError(f"axon_stop_nrt_profile rc={n}")

        return _hook

    mod = types.ModuleType("antenv.axon_hooks")
    mod.set_axon_ntff_profile_hook = lambda h: holder.__setitem__("h", h)
    mod.get_axon_ntff_profile_hook = lambda: holder.get("h")
    sys.modules["antenv.axon_hooks"] = mod
    try:
        import antenv

        antenv.axon_hooks = mod
    except ImportError:
        pass
    mod.set_axon_ntff_profile_hook(_make_hook())


# ---------------------------------------------------------------- config
N_CORES = 8
E = 800000
N_NODES = 50000
E_SHARD = E // N_CORES               # 100000
SB = 512                             # edges per superblock
NSB = (E_SHARD + SB - 1) // SB       # 196
E_PAD = NSB * SB                     # 100352
GROUPS = 16                          # 32-edge groups per superblock
BATCH = 7                            # superblocks per DMA round
NB = NSB // BATCH                    # 28 batches
NP = NB // 2                         # 14 batch pairs (DMA grain)
WBD = BATCH * 2048                   # 14336 fp16 cols per bd region
W2 = 2 * WBD                         # 28672: 2 run-interleaved regions
XW = BATCH * 512                     # 3584: x / out cols per batch
WW = BATCH * 256                     # 1792: weight cols per batch
PW = BATCH * 64                      # 448: payload cols per batch

_CACHE = {}


# ---------------------------------------------------------------- program
def _build_program(split=True):
    import concourse.bass as bass
    import concourse.mybir as mb
    from concourse.tile import TileContext

    nc = bass.Bass("TRN2", target_bir_lowering=False, debug=False,
                   num_devices=N_CORES)
    x_arr = nc.dram_tensor("x_arr", [NB, 128, XW], mb.dt.float16,
                           kind="ExternalInput")
    w_arr = nc.dram_tensor("w_arr", [NB, 128, WW], mb.dt.float16,
                           kind="ExternalInput")
    bsrc = nc.dram_tensor("bsrc", [NP, 128, 2, PW], mb.dt.float16,
                          kind="ExternalInput")
    out_dev = nc.dram_tensor("out_dev", [NB, 128, XW], mb.dt.float16,
                             kind="ExternalOutput")

    # Two persistent block-diagonal tensors, two regions (= batches) each;
    # batch pair p lands in tensor p%2. The zero slots are written once at
    # kernel start and never again (payload DMAs overwrite exactly the
    # nonzero runs, one issue per 4-partition block covering both regions).
    bds = [nc.alloc_sbuf_tensor(f"bd{k}", [128, W2], mb.dt.float16)
           for k in range(2)]

    with TileContext(nc) as tc:
        with (
            tc.tile_pool(name="xa", bufs=3) as x_pool,
            tc.tile_pool(name="wt", bufs=3) as wt_pool,
            tc.tile_pool(name="ou", bufs=3) as out_pool,
            tc.tile_pool(name="ps", bufs=4, space="PSUM") as psum_pool,
        ):
            # Stagger the zero-fills so pair 0's payload only waits on bd0:
            # DVE takes the low halves, GpSimd the high halves.
            for bdt in bds:
                nc.vector.memset(bdt.ap()[:, :WBD], 0.0)
                nc.gpsimd.memset(bdt.ap()[:, WBD:], 0.0)

            for nb in range(NB):
                p, r = nb // 2, nb % 2
                bdt = bds[p % 2]
                t = bdt.ap().tensor
                if r == 0:
                    # pair payload: block b -> bd[4b:4b+4, both regions]
                    for b in range(32):
                        dst = bass.AP(t, 4 * b * W2 + PW * b,
                                      [[W2, 4], [WBD, 2], [1, PW]])
                        eng = nc.sync if b % 2 == 0 else nc.scalar
                        eng.dma_start(out=dst, in_=bsrc[p, 4 * b:4 * b + 4])

                xt = x_pool.tile([128, XW], mb.dt.float16)
                nc.sync.dma_start(out=xt[:], in_=x_arr[nb])
                wt = wt_pool.tile([128, WW], mb.dt.float16)
                nc.scalar.dma_start(out=wt[:], in_=w_arr[nb])
                otile = out_pool.tile([128, XW], mb.dt.float16)

                for sb in range(BATCH):
                    psum = psum_pool.tile([128, 512], mb.dt.float32)
                    for g in range(GROUPS):
                        c, j = g // 4, g % 4
                        rhs = bass.AP(t, r * WBD + BATCH * g + sb,
                                      [[W2, 128], [PW, 32], [BATCH * 16, 4]])
                        nc.tensor.matmul(
                            out=psum[32 * j:32 * j + 32,
                                     128 * c:128 * c + 128],
                            lhsT=xt[:, 512 * sb + 32 * g:512 * sb + 32 * g + 32],
                            rhs=rhs,
                            start=True, stop=True,
                            tile_position=(0, 32 * j),
                        )
                    # Radial-weight multiply while draining PSUM.
                    # psum[32j+m, 128c+4b+o] * w[32j+m, 64c+2b+lo(o)]
                    ps, ww, oo = psum[:], wt[:], otile[:]
                    o0_out = bass.AP(oo.tensor, oo.offset + 512 * sb,
                                     [oo.ap[0], [128, 4], [4, 32]])
                    o0_ps = bass.AP(ps.tensor, ps.offset,
                                    [ps.ap[0], [128, 4], [4, 32]])
                    o0_w = bass.AP(ww.tensor, ww.offset + 256 * sb,
                                   [ww.ap[0], [64, 4], [2, 32]])
                    nc.vector.tensor_mul(o0_out, o0_ps, o0_w)
                    o1_out = bass.AP(oo.tensor, oo.offset + 512 * sb + 1,
                                     [oo.ap[0], [128, 4], [4, 32], [1, 3]])
                    o1_ps = bass.AP(ps.tensor, ps.offset + 1,
                                    [ps.ap[0], [128, 4], [4, 32], [1, 3]])
                    o1_w = bass.AP(ww.tensor, ww.offset + 256 * sb + 1,
                                   [ww.ap[0], [64, 4], [2, 32], [0, 3]])
                    nc.vector.tensor_mul(o1_out, o1_ps, o1_w)

                nc.scalar.dma_start(out=out_dev[nb], in_=otile[:])

    if split:
        _split_waits(nc)
    return nc


# ---------------------------------------------------------------- host side
def _host_prep(basis, edge_weights, node_features, U):
    nf16 = np.ascontiguousarray(node_features).astype(np.float16)

    in_maps = []
    for core in range(N_CORES):
        lo = core * E_SHARD
        hi = lo + E_SHARD
        u = np.zeros((E_PAD,), np.int64)
        u[:E_SHARD] = U[lo:hi]

        # x_arr[nb, 4b+i, 512*sb + 32g+m], edge e = s*512 + g*32 + b,
        # s = nb*BATCH + sb
        xg = nf16[u]                                    # [E_PAD, 4, 32]
        xa = xg.reshape(NB, BATCH, GROUPS, 32, 4, 32)   # [nb,sb,g,b,i,m]
        xa = xa.transpose(0, 3, 4, 1, 2, 5)             # [nb,b,i,sb,g,m]
        x_arr = np.ascontiguousarray(
            xa.reshape(NB, 128, XW), np.float16)

        # w_arr[nb, 32j+m, 256*sb + 64c+2b+l]
        w = np.zeros((E_PAD, 2, 32), np.float16)
        w[:E_SHARD] = edge_weights[lo:hi].astype(np.float16)
        ws = w.reshape(NB, BATCH, 4, 4, 32, 2, 32)      # [nb,sb,c,j,b,l,m]
        ws = ws.transpose(0, 3, 6, 1, 2, 4, 5)          # [nb,j,m,sb,c,b,l]
        w_arr = np.ascontiguousarray(
            ws.reshape(NB, 128, WW), np.float16)

        # bsrc[p, 4b+i, r, 112o + 7g + sb] = basis[e, o, i], nb = 2p + r
        b = np.zeros((E_PAD, 4, 4), np.float16)
        b[:E_SHARD] = basis[lo:hi].astype(np.float16)
        bs = b.reshape(NP, 2, BATCH, GROUPS, 32, 4, 4)  # [p,r,sb,g,b,o,i]
        bs = bs.transpose(0, 4, 6, 1, 5, 3, 2)          # [p,b,i,r,o,g,sb]
        bsrc = np.ascontiguousarray(
            bs.reshape(NP, 128, 2, PW), np.float16)

        in_maps.append({"x_arr": x_arr, "w_arr": w_arr, "bsrc": bsrc})
    return in_maps


def _unshard(results):
    outs = []
    for core in range(N_CORES):
        od = results[core]["out_dev"]                   # [NB, 128, XW] fp16
        o7 = od.reshape(NB, 4, 32, BATCH, 4, 32, 4)     # [nb,j,m,sb,c,b,o]
        o7 = o7.transpose(0, 3, 4, 1, 5, 6, 2)          # [nb,sb,c,j,b,o,m]
        outs.append(
            o7.reshape(E_PAD, 4, 32)[:E_SHARD].astype(np.float32))
    return np.concatenate(outs, axis=0)


# ---------------------------------------------------------------- entry
def kernel(basis, edge_weights, node_features, U, _trace=False):
    """Full inputs -> full output. Shards over 8 NeuronCores internally."""
    basis = np.asarray(basis, dtype=np.float32)
    edge_weights = np.asarray(edge_weights, dtype=np.float32)
    node_features = np.asarray(node_features, dtype=np.float32)
    U = np.asarray(U)

    _apply_tile_patch()
    _install_axon_ntff_hook()
    from concourse.bass_utils import run_bass_kernel_spmd

    if "nc" not in _CACHE:
        _CACHE["nc"] = _build_program()
    nc = _CACHE["nc"]

    in_maps = _host_prep(basis, edge_weights, node_features, U)
    res = run_bass_kernel_spmd(nc, in_maps, core_ids=list(range(N_CORES)),
                               trace=_trace)
    out = _unshard(res.results)
    if _trace:
        return out, res
    return out


# revision 15
# speedup vs baseline: 1.0097x; 1.0097x over previous
"""Equivariant matmul kernel for Trainium2 (8 NeuronCores, Bass/Tile).

Problem (per edge e of E=800000):
    out[e,o,m] = (sum_i basis[e,o,i] * node_features[U[e],i,m]) * w[e,lo(o),m]
with D_IN=D_OUT=4, M=32, lo = [0,1,1,1].

Strategy (edge-parallel sharding, 100k edges/core, fp16 data / fp32 PSUM):
- Host prep per core shard (196 superblocks x 512 edges, batched 7 SBs
  per DMA round -> 28 batches):
    * x_arr[nb, 4b+i, 512*sb + 32g+m] = nf[U[e],i,m]  (gather, fp16)
    * w_arr[nb, 32j+m, 256*sb + 64c+2b+l] = edge_weights (compact, fp16)
    * bsrc[nb, 4b+i, 448b-relative run (o,g,sb)] = basis^T payload (fp16)
- Device per batch:
    * one plain DMA each for x / w / payload / out; the payload lands in
      a persistent pre-zeroed block-diagonal tensor bd[4b+i, 448b+112o+
      7g+sb] via a partition-crossing strided AP (512B+ contiguous runs)
    * per superblock 16 fp16 TensorE matmuls (K=128, M=32, N=128,
      4x col-tiled) read bd through a strided AP; PSUM accumulates fp32
    * 2 DVE tensor_muls apply the radial weights (stride-0 broadcast
      expands the l=1 weight over o in {1,2,3}) draining PSUM->SBUF fp16
- Host post: invert the layout permutation, cast fp32, concat shards.
"""

import contextlib
import ctypes
import sys
import types

import numpy as np

# ---------------------------------------------------------------- harness
# Workaround for walrus "Too many sync wait commands": this container's
# compiler accepts at most MAXW sem-waits per instruction; Tile emits more
# on the tail drain and occasionally mid-kernel. Split extras onto NOPs.
MAXW = 1


def _apply_tile_patch():
    import concourse.tile as tile_mod
    import concourse.mybir as mb
    from concourse.vector_clock import ScopedClock

    def _patched_drain_and_barrier(self, tick_clock, wait_clock):
        nc = self.nc
        drain_inst = nc.sync.drain()
        wait_clock.add_sem_waits(
            drain_inst.ins, ScopedClock({None: tick_clock.global_clock})
        )
        si = drain_inst.ins.sync_info
        if si is not None and len(si.on_wait) > 1:
            extra = list(si.on_wait[1:])
            si.on_wait = si.on_wait[:1]
            for w in extra:
                nop = nc.sync.nop(nofuse=True, hint="split_drain_wait")
                nop.ins.sync_info = mb.SyncInfo(on_wait=[w], on_update=[])
        nc.all_engine_barrier()
        assert self.sems is not None
        popped = nc._tile_sem_poison_stack.pop()
        assert popped is self._sem_poison
        nc.clear_and_free_semaphores(list(self.sems.allocated().values()))
        nc.all_engine_barrier()

    tile_mod.TileContext._drain_and_barrier = _patched_drain_and_barrier


_nop_counter = [0]


def _split_waits(nc, maxw=MAXW):
    import concourse.mybir as mb

    n_split = 0
    for fn in nc.m.functions:
        for blk in fn.blocks:
            insts = list(blk.instructions)
            out = []
            changed = False
            for inst in insts:
                si = getattr(inst, "sync_info", None)
                if si is not None and si.on_wait is not None and len(si.on_wait) > maxw:
                    extra = list(si.on_wait[:-maxw])
                    si.on_wait = list(si.on_wait[-maxw:])
                    for w in extra:
                        _nop_counter[0] += 1
                        nop = mb.InstNoOp(
                            name=f"waitsplit-{_nop_counter[0]}",
                            ins=[], outs=[], engine=inst.engine,
                        )
                        nop.sync_info = mb.SyncInfo(on_wait=[w], on_update=[])
                        out.append(nop)
                        n_split += 1
                    changed = True
                out.append(inst)
            if changed:
                blk.instructions = out
    return n_split


def _install_axon_ntff_hook():
    """Register the NTFF profile hook the agent image's antenv lacks, so
    run_bass_kernel_spmd(trace=True) can report HW exec time."""
    if "antenv.axon_hooks" in sys.modules:
        return
    so_path = "/opt/axon/libaxon_pjrt.so"
    holder = {}

    def _make_hook():
        try:
            lib = ctypes.CDLL(so_path)
        except OSError:
            return None
        if not hasattr(lib, "axon_start_nrt_profile"):
            return None
        lib.axon_start_nrt_profile.argtypes = [
            ctypes.POINTER(ctypes.c_int64), ctypes.c_size_t,
        ]
        lib.axon_start_nrt_profile.restype = ctypes.c_int64
        lib.axon_stop_nrt_profile.argtypes = [ctypes.c_char_p]
        lib.axon_stop_nrt_profile.restype = ctypes.c_int64

        @contextlib.contextmanager
        def _hook(output_dir, device_ids):
            import jax

            jax.devices()
            if device_ids:
                ids = (ctypes.c_int64 * len(device_ids))(*device_ids)
                rc = lib.axon_start_nrt_profile(ids, len(device_ids))
            else:
                rc = lib.axon_start_nrt_profile(None, 0)
            if rc != 0:
                raise RuntimeError(f"axon_start_nrt_profile rc={rc}")
            try:
                yield
            finally:
                n = lib.axon_stop_nrt_profile(str(output_dir).encode())
                if n < 0:
                    raise RuntimeError(f"axon_stop_nrt_profile rc={n}")

        return _hook

    mod = types.ModuleType("antenv.axon_hooks")
    mod.set_axon_ntff_profile_hook = lambda h: holder.__setitem__("h", h)
    mod.get_axon_ntff_profile_hook = lambda: holder.get("h")
    sys.modules["antenv.axon_hooks"] = mod
    try:
        import antenv

        antenv.axon_hooks = mod
    except ImportError:
        pass
    mod.set_axon_ntff_profile_hook(_make_hook())


# ---------------------------------------------------------------- config
N_CORES = 8
E = 800000
N_NODES = 50000
E_SHARD = E // N_CORES               # 100000
SB = 512                             # edges per superblock
NSB = (E_SHARD + SB - 1) // SB       # 196
E_PAD = NSB * SB                     # 100352
GROUPS = 16                          # 32-edge groups per superblock
BATCH = 7                            # superblocks per DMA round
NB = NSB // BATCH                    # 28 batches
NP = NB // 2                         # 14 batch pairs (DMA grain)
WBD = BATCH * 2048                   # 14336 fp16 cols per bd region
W2 = 2 * WBD                         # 28672: 2 run-interleaved regions
XW = BATCH * 512                     # 3584: x / out cols per batch
WW = BATCH * 256                     # 1792: weight cols per batch
PW = BATCH * 64                      # 448: payload cols per batch

_CACHE = {}


# ---------------------------------------------------------------- program
def _build_program(split=True):
    import concourse.bass as bass
    import concourse.mybir as mb
    from concourse.tile import TileContext

    nc = bass.Bass("TRN2", target_bir_lowering=False, debug=False,
                   num_devices=N_CORES)
    x_arr = nc.dram_tensor("x_arr", [NB, 128, XW], mb.dt.float16,
                           kind="ExternalInput")
    w_arr = nc.dram_tensor("w_arr", [NB, 128, WW], mb.dt.float16,
                           kind="ExternalInput")
    bsrc = nc.dram_tensor("bsrc", [NP, 128, 2, PW], mb.dt.float16,
                          kind="ExternalInput")
    out_dev = nc.dram_tensor("out_dev", [NB, 128, XW], mb.dt.float16,
                             kind="ExternalOutput")

    # Two persistent block-diagonal tensors, two regions (= batches) each;
    # batch pair p lands in tensor p%2. The zero slots are written once at
    # kernel start and never again (payload DMAs overwrite exactly the
    # nonzero runs, one issue per 4-partition block covering both regions).
    bds = [nc.alloc_sbuf_tensor(f"bd{k}", [128, W2], mb.dt.float16)
           for k in range(2)]

    with TileContext(nc) as tc:
        with (
            tc.tile_pool(name="xa", bufs=3) as x_pool,
            tc.tile_pool(name="wt", bufs=3) as wt_pool,
            tc.tile_pool(name="ou", bufs=3) as out_pool,
            tc.tile_pool(name="ps", bufs=2, space="PSUM") as psum_pool,
        ):
            # Stagger the zero-fills so pair 0's payload only waits on bd0:
            # DVE takes the low halves, GpSimd the high halves.
            for bdt in bds:
                nc.vector.memset(bdt.ap()[:, :WBD], 0.0)
                nc.gpsimd.memset(bdt.ap()[:, WBD:], 0.0)

            for nb in range(NB):
                p, r = nb // 2, nb % 2
                bdt = bds[p % 2]
                t = bdt.ap().tensor
                if r == 0:
                    # pair payload: block b -> bd[4b:4b+4, both regions]
                    for b in range(32):
                        dst = bass.AP(t, 4 * b * W2 + PW * b,
                                      [[W2, 4], [WBD, 2], [1, PW]])
                        eng = nc.sync if b % 2 == 0 else nc.scalar
                        eng.dma_start(out=dst, in_=bsrc[p, 4 * b:4 * b + 4])

                xt = x_pool.tile([128, XW], mb.dt.float16)
                nc.sync.dma_start(out=xt[:], in_=x_arr[nb])
                wt = wt_pool.tile([128, WW], mb.dt.float16)
                nc.scalar.dma_start(out=wt[:], in_=w_arr[nb])
                otile = out_pool.tile([128, XW], mb.dt.float16)

                for sbp in range(4):
                    # 3 two-superblock PSUM tiles (2 banks) + 1 single
                    sb0 = 2 * sbp
                    ns = 2 if sbp < 3 else 1
                    psum = psum_pool.tile([128, 512 * ns], mb.dt.float32,
                                          tag=f"p{ns}")
                    for s in range(ns):
                        sb = sb0 + s
                        for g in range(GROUPS):
                            c, j = g // 4, g % 4
                            rhs = bass.AP(t, r * WBD + BATCH * g + sb,
                                          [[W2, 128], [PW, 32],
                                           [BATCH * 16, 4]])
                            nc.tensor.matmul(
                                out=psum[32 * j:32 * j + 32,
                                         512 * s + 128 * c:
                                         512 * s + 128 * c + 128],
                                lhsT=xt[:, 512 * sb + 32 * g:
                                        512 * sb + 32 * g + 32],
                                rhs=rhs,
                                start=True, stop=True,
                                tile_position=(0, 32 * j),
                            )
                    # Radial-weight multiply while draining PSUM (both
                    # superblocks in one op): psum[32j+m, 512s+128c+4b+o]
                    # * w[32j+m, 256s+64c+2b+lo(o)]
                    ps, ww, oo = psum[:], wt[:], otile[:]
                    d0 = [[512, ns], [128, 4], [4, 32]]
                    dw = [[256, ns], [64, 4], [2, 32]]
                    o0_out = bass.AP(oo.tensor, oo.offset + 512 * sb0,
                                     [oo.ap[0]] + d0)
                    o0_ps = bass.AP(ps.tensor, ps.offset, [ps.ap[0]] + d0)
                    o0_w = bass.AP(ww.tensor, ww.offset + 256 * sb0,
                                   [ww.ap[0]] + dw)
                    nc.vector.tensor_mul(o0_out, o0_ps, o0_w)
                    o1_out = bass.AP(oo.tensor, oo.offset + 512 * sb0 + 1,
                                     [oo.ap[0]] + d0 + [[1, 3]])
                    o1_ps = bass.AP(ps.tensor, ps.offset + 1,
                                    [ps.ap[0]] + d0 + [[1, 3]])
                    o1_w = bass.AP(ww.tensor, ww.offset + 256 * sb0 + 1,
                                   [ww.ap[0]] + dw + [[0, 3]])
                    nc.vector.tensor_mul(o1_out, o1_ps, o1_w)

                nc.scalar.dma_start(out=out_dev[nb], in_=otile[:])

    if split:
        _split_waits(nc)
    return nc


# ---------------------------------------------------------------- host side
def _host_prep(basis, edge_weights, node_features, U):
    nf16 = np.ascontiguousarray(node_features).astype(np.float16)

    in_maps = []
    for core in range(N_CORES):
        lo = core * E_SHARD
        hi = lo + E_SHARD
        u = np.zeros((E_PAD,), np.int64)
        u[:E_SHARD] = U[lo:hi]

        # x_arr[nb, 4b+i, 512*sb + 32g+m], edge e = s*512 + g*32 + b,
        # s = nb*BATCH + sb
        xg = nf16[u]                                    # [E_PAD, 4, 32]
        xa = xg.reshape(NB, BATCH, GROUPS, 32, 4, 32)   # [nb,sb,g,b,i,m]
        xa = xa.transpose(0, 3, 4, 1, 2, 5)             # [nb,b,i,sb,g,m]
        x_arr = np.ascontiguousarray(
            xa.reshape(NB, 128, XW), np.float16)

        # w_arr[nb, 32j+m, 256*sb + 64c+2b+l]
        w = np.zeros((E_PAD, 2, 32), np.float16)
        w[:E_SHARD] = edge_weights[lo:hi].astype(np.float16)
        ws = w.reshape(NB, BATCH, 4, 4, 32, 2, 32)      # [nb,sb,c,j,b,l,m]
        ws = ws.transpose(0, 3, 6, 1, 2, 4, 5)          # [nb,j,m,sb,c,b,l]
        w_arr = np.ascontiguousarray(
            ws.reshape(NB, 128, WW), np.float16)

        # bsrc[p, 4b+i, r, 112o + 7g + sb] = basis[e, o, i], nb = 2p + r
        b = np.zeros((E_PAD, 4, 4), np.float16)
        b[:E_SHARD] = basis[lo:hi].astype(np.float16)
        bs = b.reshape(NP, 2, BATCH, GROUPS, 32, 4, 4)  # [p,r,sb,g,b,o,i]
        bs = bs.transpose(0, 4, 6, 1, 5, 3, 2)          # [p,b,i,r,o,g,sb]
        bsrc = np.ascontiguousarray(
            bs.reshape(NP, 128, 2, PW), np.float16)

        in_maps.append({"x_arr": x_arr, "w_arr": w_arr, "bsrc": bsrc})
    return in_maps


def _unshard(results):
    outs = []
    for core in range(N_CORES):
        od = results[core]["out_dev"]                   # [NB, 128, XW] fp16
        o7 = od.reshape(NB, 4, 32, BATCH, 4, 32, 4)     # [nb,j,m,sb,c,b,o]
        o7 = o7.transpose(0, 3, 4, 1, 5, 6, 2)          # [nb,sb,c,j,b,o,m]
        outs.append(
            o7.reshape(E_PAD, 4, 32)[:E_SHARD].astype(np.float32))
    return np.concatenate(outs, axis=0)


# ---------------------------------------------------------------- entry
def kernel(basis, edge_weights, node_features, U, _trace=False):
    """Full inputs -> full output. Shards over 8 NeuronCores internally."""
    basis = np.asarray(basis, dtype=np.float32)
    edge_weights = np.asarray(edge_weights, dtype=np.float32)
    node_features = np.asarray(node_features, dtype=np.float32)
    U = np.asarray(U)

    _apply_tile_patch()
    _install_axon_ntff_hook()
    from concourse.bass_utils import run_bass_kernel_spmd

    if "nc" not in _CACHE:
        _CACHE["nc"] = _build_program()
    nc = _CACHE["nc"]

    in_maps = _host_prep(basis, edge_weights, node_features, U)
    res = run_bass_kernel_spmd(nc, in_maps, core_ids=list(range(N_CORES)),
                               trace=_trace)
    out = _unshard(res.results)
    if _trace:
        return out, res
    return out


# revision 16
# speedup vs baseline: 1.0397x; 1.0298x over previous
"""Equivariant matmul kernel for Trainium2 (8 NeuronCores, Bass/Tile).

Problem (per edge e of E=800000):
    out[e,o,m] = (sum_i basis[e,o,i] * node_features[U[e],i,m]) * w[e,lo(o),m]
with D_IN=D_OUT=4, M=32, lo = [0,1,1,1].

Strategy (edge-parallel sharding, 100k edges/core, fp16 data / fp32 PSUM):
- Host prep per core shard (196 superblocks x 512 edges, batched 7 SBs
  per DMA round -> 28 batches):
    * x_arr[nb, 4b+i, 512*sb + 32g+m] = nf[U[e],i,m]  (gather, fp16)
    * w_arr[nb, 32j+m, 256*sb + 64c+2b+l] = edge_weights (compact, fp16)
    * bsrc[nb, 4b+i, 448b-relative run (o,g,sb)] = basis^T payload (fp16)
- Device per batch:
    * one plain DMA each for x / w / payload / out; the payload lands in
      a persistent pre-zeroed block-diagonal tensor bd[4b+i, 448b+112o+
      7g+sb] via a partition-crossing strided AP (512B+ contiguous runs)
    * per superblock 16 fp16 TensorE matmuls (K=128, M=32, N=128,
      4x col-tiled) read bd through a strided AP; PSUM accumulates fp32
    * 2 DVE tensor_muls apply the radial weights (stride-0 broadcast
      expands the l=1 weight over o in {1,2,3}) draining PSUM->SBUF fp16
- Host post: invert the layout permutation, cast fp32, concat shards.
"""

import contextlib
import ctypes
import sys
import types

import numpy as np

# ---------------------------------------------------------------- harness
# Workaround for walrus "Too many sync wait commands": this container's
# compiler accepts at most MAXW sem-waits per instruction; Tile emits more
# on the tail drain and occasionally mid-kernel. Split extras onto NOPs.
MAXW = 1


def _apply_tile_patch():
    import concourse.tile as tile_mod
    import concourse.mybir as mb
    from concourse.vector_clock import ScopedClock

    def _patched_drain_and_barrier(self, tick_clock, wait_clock):
        nc = self.nc
        drain_inst = nc.sync.drain()
        wait_clock.add_sem_waits(
            drain_inst.ins, ScopedClock({None: tick_clock.global_clock})
        )
        si = drain_inst.ins.sync_info
        if si is not None and len(si.on_wait) > 1:
            extra = list(si.on_wait[1:])
            si.on_wait = si.on_wait[:1]
            for w in extra:
                nop = nc.sync.nop(nofuse=True, hint="split_drain_wait")
                nop.ins.sync_info = mb.SyncInfo(on_wait=[w], on_update=[])
        nc.all_engine_barrier()
        assert self.sems is not None
        popped = nc._tile_sem_poison_stack.pop()
        assert popped is self._sem_poison
        nc.clear_and_free_semaphores(list(self.sems.allocated().values()))
        nc.all_engine_barrier()

    tile_mod.TileContext._drain_and_barrier = _patched_drain_and_barrier


_nop_counter = [0]


def _split_waits(nc, maxw=MAXW):
    import concourse.mybir as mb

    n_split = 0
    for fn in nc.m.functions:
        for blk in fn.blocks:
            insts = list(blk.instructions)
            out = []
            changed = False
            for inst in insts:
                si = getattr(inst, "sync_info", None)
                if si is not None and si.on_wait is not None and len(si.on_wait) > maxw:
                    extra = list(si.on_wait[:-maxw])
                    si.on_wait = list(si.on_wait[-maxw:])
                    for w in extra:
                        _nop_counter[0] += 1
                        nop = mb.InstNoOp(
                            name=f"waitsplit-{_nop_counter[0]}",
                            ins=[], outs=[], engine=inst.engine,
                        )
                        nop.sync_info = mb.SyncInfo(on_wait=[w], on_update=[])
                        out.append(nop)
                        n_split += 1
                    changed = True
                out.append(inst)
            if changed:
                blk.instructions = out
    return n_split


def _install_axon_ntff_hook():
    """Register the NTFF profile hook the agent image's antenv lacks, so
    run_bass_kernel_spmd(trace=True) can report HW exec time."""
    if "antenv.axon_hooks" in sys.modules:
        return
    so_path = "/opt/axon/libaxon_pjrt.so"
    holder = {}

    def _make_hook():
        try:
            lib = ctypes.CDLL(so_path)
        except OSError:
            return None
        if not hasattr(lib, "axon_start_nrt_profile"):
            return None
        lib.axon_start_nrt_profile.argtypes = [
            ctypes.POINTER(ctypes.c_int64), ctypes.c_size_t,
        ]
        lib.axon_start_nrt_profile.restype = ctypes.c_int64
        lib.axon_stop_nrt_profile.argtypes = [ctypes.c_char_p]
        lib.axon_stop_nrt_profile.restype = ctypes.c_int64

        @contextlib.contextmanager
        def _hook(output_dir, device_ids):
            import jax

            jax.devices()
            if device_ids:
                ids = (ctypes.c_int64 * len(device_ids))(*device_ids)
                rc = lib.axon_start_nrt_profile(ids, len(device_ids))
            else:
                rc = lib.axon_start_nrt_profile(None, 0)
            if rc != 0:
                raise RuntimeError(f"axon_start_nrt_profile rc={rc}")
            try:
                yield
            finally:
                n = lib.axon_stop_nrt_profile(str(output_dir).encode())
                if n < 0:
                    raise RuntimeError(f"axon_stop_nrt_profile rc={n}")

        return _hook

    mod = types.ModuleType("antenv.axon_hooks")
    mod.set_axon_ntff_profile_hook = lambda h: holder.__setitem__("h", h)
    mod.get_axon_ntff_profile_hook = lambda: holder.get("h")
    sys.modules["antenv.axon_hooks"] = mod
    try:
        import antenv

        antenv.axon_hooks = mod
    except ImportError:
        pass
    mod.set_axon_ntff_profile_hook(_make_hook())


# ---------------------------------------------------------------- config
N_CORES = 8
E = 800000
N_NODES = 50000
E_SHARD = E // N_CORES               # 100000
SB = 512                             # edges per superblock
NSB = (E_SHARD + SB - 1) // SB       # 196
E_PAD = NSB * SB                     # 100352
GROUPS = 16                          # 32-edge groups per superblock
BATCH = 7                            # superblocks per DMA round
NB = NSB // BATCH                    # 28 batches
NP = NB // 2                         # 14 batch pairs (DMA grain)
WBD = BATCH * 2048                   # 14336 fp16 cols per bd region
W2 = 2 * WBD                         # 28672: 2 run-interleaved regions
XW = BATCH * 512                     # 3584: x / out cols per batch
WW = BATCH * 256                     # 1792: weight cols per batch
PW = BATCH * 64                      # 448: payload cols per batch

_CACHE = {}


# ---------------------------------------------------------------- program
def _build_program(split=True):
    import concourse.bass as bass
    import concourse.mybir as mb
    from concourse.tile import TileContext

    nc = bass.Bass("TRN2", target_bir_lowering=False, debug=False,
                   num_devices=N_CORES)
    x_arr = nc.dram_tensor("x_arr", [NB, 128, XW], mb.dt.float16,
                           kind="ExternalInput")
    w_arr = nc.dram_tensor("w_arr", [NB, 128, WW], mb.dt.float16,
                           kind="ExternalInput")
    bsrc = nc.dram_tensor("bsrc", [NP, 128, 2, PW], mb.dt.float16,
                          kind="ExternalInput")
    out_dev = nc.dram_tensor("out_dev", [NB, 128, XW], mb.dt.float16,
                             kind="ExternalOutput")

    # Two persistent block-diagonal tensors, two regions (= batches) each;
    # batch pair p lands in tensor p%2. The zero slots are written once at
    # kernel start and never again (payload DMAs overwrite exactly the
    # nonzero runs, one issue per 4-partition block covering both regions).
    bds = [nc.alloc_sbuf_tensor(f"bd{k}", [128, W2], mb.dt.float16)
           for k in range(2)]

    with TileContext(nc) as tc:
        with (
            tc.tile_pool(name="xa", bufs=3) as x_pool,
            tc.tile_pool(name="wt", bufs=3) as wt_pool,
            tc.tile_pool(name="ou", bufs=3) as out_pool,
            tc.tile_pool(name="ps", bufs=2, space="PSUM") as psum_pool,
        ):
            # Stagger the zero-fills so pair 0's payload only waits on bd0:
            # DVE takes the low halves, GpSimd the high halves.
            for bdt in bds:
                nc.vector.memset(bdt.ap()[:, :WBD], 0.0)
                nc.gpsimd.memset(bdt.ap()[:, WBD:], 0.0)

            for nb in range(NB):
                p, r = nb // 2, nb % 2
                bdt = bds[p % 2]
                t = bdt.ap().tensor
                if r == 0:
                    # pair payload: block b -> bd[4b:4b+4, both regions]
                    for b in range(32):
                        dst = bass.AP(t, 4 * b * W2 + PW * b,
                                      [[W2, 4], [WBD, 2], [1, PW]])
                        eng = nc.sync if b % 2 == 0 else nc.scalar
                        eng.dma_start(out=dst, in_=bsrc[p, 4 * b:4 * b + 4])

                xt = x_pool.tile([128, XW], mb.dt.float16)
                nc.sync.dma_start(out=xt[:], in_=x_arr[nb])
                wt = wt_pool.tile([128, WW], mb.dt.float16)
                nc.scalar.dma_start(out=wt[:], in_=w_arr[nb])
                otile = out_pool.tile([128, XW], mb.dt.float16)

                for sbp in range(4):
                    # 3 two-superblock PSUM tiles (2 banks) + 1 single
                    sb0 = 2 * sbp
                    ns = 2 if sbp < 3 else 1
                    psum = psum_pool.tile([128, 512 * ns], mb.dt.float32,
                                          tag=f"p{ns}", bufs=3 if ns == 2 else 2)
                    for s in range(ns):
                        sb = sb0 + s
                        for g in range(GROUPS):
                            c, j = g // 4, g % 4
                            rhs = bass.AP(t, r * WBD + BATCH * g + sb,
                                          [[W2, 128], [PW, 32],
                                           [BATCH * 16, 4]])
                            nc.tensor.matmul(
                                out=psum[32 * j:32 * j + 32,
                                         512 * s + 128 * c:
                                         512 * s + 128 * c + 128],
                                lhsT=xt[:, 512 * sb + 32 * g:
                                        512 * sb + 32 * g + 32],
                                rhs=rhs,
                                start=True, stop=True,
                                tile_position=(0, 32 * j),
                            )
                    # Radial-weight multiply while draining PSUM (both
                    # superblocks in one op): psum[32j+m, 512s+128c+4b+o]
                    # * w[32j+m, 256s+64c+2b+lo(o)]
                    ps, ww, oo = psum[:], wt[:], otile[:]
                    d0 = [[512, ns], [128, 4], [4, 32]]
                    dw = [[256, ns], [64, 4], [2, 32]]
                    o0_out = bass.AP(oo.tensor, oo.offset + 512 * sb0,
                                     [oo.ap[0]] + d0)
                    o0_ps = bass.AP(ps.tensor, ps.offset, [ps.ap[0]] + d0)
                    o0_w = bass.AP(ww.tensor, ww.offset + 256 * sb0,
                                   [ww.ap[0]] + dw)
                    nc.vector.tensor_mul(o0_out, o0_ps, o0_w)
                    o1_out = bass.AP(oo.tensor, oo.offset + 512 * sb0 + 1,
                                     [oo.ap[0]] + d0 + [[1, 3]])
                    o1_ps = bass.AP(ps.tensor, ps.offset + 1,
                                    [ps.ap[0]] + d0 + [[1, 3]])
                    o1_w = bass.AP(ww.tensor, ww.offset + 256 * sb0 + 1,
                                   [ww.ap[0]] + dw + [[0, 3]])
                    nc.vector.tensor_mul(o1_out, o1_ps, o1_w)

                nc.scalar.dma_start(out=out_dev[nb], in_=otile[:])

    if split:
        _split_waits(nc)
    return nc


# ---------------------------------------------------------------- host side
def _host_prep(basis, edge_weights, node_features, U):
    nf16 = np.ascontiguousarray(node_features).astype(np.float16)

    in_maps = []
    for core in range(N_CORES):
        lo = core * E_SHARD
        hi = lo + E_SHARD
        u = np.zeros((E_PAD,), np.int64)
        u[:E_SHARD] = U[lo:hi]

        # x_arr[nb, 4b+i, 512*sb + 32g+m], edge e = s*512 + g*32 + b,
        # s = nb*BATCH + sb
        xg = nf16[u]                                    # [E_PAD, 4, 32]
        xa = xg.reshape(NB, BATCH, GROUPS, 32, 4, 32)   # [nb,sb,g,b,i,m]
        xa = xa.transpose(0, 3, 4, 1, 2, 5)             # [nb,b,i,sb,g,m]
        x_arr = np.ascontiguousarray(
            xa.reshape(NB, 128, XW), np.float16)

        # w_arr[nb, 32j+m, 256*sb + 64c+2b+l]
        w = np.zeros((E_PAD, 2, 32), np.float16)
        w[:E_SHARD] = edge_weights[lo:hi].astype(np.float16)
        ws = w.reshape(NB, BATCH, 4, 4, 32, 2, 32)      # [nb,sb,c,j,b,l,m]
        ws = ws.transpose(0, 3, 6, 1, 2, 4, 5)          # [nb,j,m,sb,c,b,l]
        w_arr = np.ascontiguousarray(
            ws.reshape(NB, 128, WW), np.float16)

        # bsrc[p, 4b+i, r, 112o + 7g + sb] = basis[e, o, i], nb = 2p + r
        b = np.zeros((E_PAD, 4, 4), np.float16)
        b[:E_SHARD] = basis[lo:hi].astype(np.float16)
        bs = b.reshape(NP, 2, BATCH, GROUPS, 32, 4, 4)  # [p,r,sb,g,b,o,i]
        bs = bs.transpose(0, 4, 6, 1, 5, 3, 2)          # [p,b,i,r,o,g,sb]
        bsrc = np.ascontiguousarray(
            bs.reshape(NP, 128, 2, PW), np.float16)

        in_maps.append({"x_arr": x_arr, "w_arr": w_arr, "bsrc": bsrc})
    return in_maps


def _unshard(results):
    outs = []
    for core in range(N_CORES):
        od = results[core]["out_dev"]                   # [NB, 128, XW] fp16
        o7 = od.reshape(NB, 4, 32, BATCH, 4, 32, 4)     # [nb,j,m,sb,c,b,o]
        o7 = o7.transpose(0, 3, 4, 1, 5, 6, 2)          # [nb,sb,c,j,b,o,m]
        outs.append(
            o7.reshape(E_PAD, 4, 32)[:E_SHARD].astype(np.float32))
    return np.concatenate(outs, axis=0)


# ---------------------------------------------------------------- entry
def kernel(basis, edge_weights, node_features, U, _trace=False):
    """Full inputs -> full output. Shards over 8 NeuronCores internally."""
    basis = np.asarray(basis, dtype=np.float32)
    edge_weights = np.asarray(edge_weights, dtype=np.float32)
    node_features = np.asarray(node_features, dtype=np.float32)
    U = np.asarray(U)

    _apply_tile_patch()
    _install_axon_ntff_hook()
    from concourse.bass_utils import run_bass_kernel_spmd

    if "nc" not in _CACHE:
        _CACHE["nc"] = _build_program()
    nc = _CACHE["nc"]

    in_maps = _host_prep(basis, edge_weights, node_features, U)
    res = run_bass_kernel_spmd(nc, in_maps, core_ids=list(range(N_CORES)),
                               trace=_trace)
    out = _unshard(res.results)
    if _trace:
        return out, res
    return out
